# revision 35
# baseline (speedup 1.0000x reference)
"""Trainium2 Bass kernel for a 2-layer linear-attention transformer.

Sharding: 8 cores = 2 batches x 4 sequence segments (512 rows each).
Each core runs the full per-token pipeline on its rows; the only
cross-core dependency is the causal linear-attention prefix state,
exchanged once per layer via a 4-rank AllGather.

On-chip layout: activations are kept feature-major ("transposed",
feature dim on SBUF partitions) so every matmul contracts the partition
dim with no activation transposes.

Schedule (v1): per layer the k/v projections, per-head prefix states and
the AllGather all run BEFORE the q projection, so the collective is
hidden under the q projection + local score matmuls.  The per-head
numerators accumulate in a dedicated PSUM pool so heads pipeline across
the prefix-state wait.  Weight strips double-buffer across phases.
"""

import sys

for _p in ("/opt/trn_rl_repo", "/root/.axon_site/_ro/trn_rl_repo"):
    if _p not in sys.path:
        sys.path.append(_p)

import numpy as np

import concourse.bass as bass
import concourse.mybir as mybir
import concourse.tile as tile
from concourse import bacc, bass_isa
from concourse.bass_utils import run_bass_kernel_spmd
from concourse.masks import make_identity

F32 = mybir.dt.float32
F32R = mybir.dt.float32r
BF16 = mybir.dt.bfloat16
AF = mybir.ActivationFunctionType
OP = mybir.AluOpType


class Cfg:
    def __init__(self, D=1024, H=16, FF=4096, R=512, depth=2, n_cores=8, segs=4,
                 use_f32r=True, act_bf16=True, warm_every=0, warm_cols=512):
        self.D, self.H, self.FF, self.R, self.depth = D, H, FF, R, depth
        self.n_cores, self.segs = n_cores, segs
        self.B = n_cores // segs
        self.dh = D // H
        self.P = 128
        self.KT = D // 128          # k-tiles over D
        self.NB = R // 128          # row blocks per core
        self.FB = FF // 128         # ff blocks
        self.use_f32r = use_f32r
        self.act_bf16 = act_bf16
        self.warm_every = warm_every
        self.warm_cols = warm_cols
        assert self.dh == 64 and self.R % 128 == 0 and self.D % 128 == 0


def build_program(cfg: Cfg):
    """Build the SPMD Bass program."""
    nc = bacc.Bacc("TRN2", target_bir_lowering=False, debug=False,
                   num_devices=cfg.n_cores)
    D, FF, R, P = cfg.D, cfg.FF, cfg.R, cfg.P
    depth = cfg.depth

    MMDT = BF16 if cfg.act_bf16 else (F32R if cfg.use_f32r else F32)
    io = {}
    io["xT"] = nc.dram_tensor("xT", [D, R], F32, kind="ExternalInput").ap()
    wnames = {"Wq", "Wk", "Wv", "Wo", "W1", "W2"}
    for nm, shp in (("Wq", [depth, D, D]), ("Wk", [depth, D, D]),
                    ("Wv", [depth, D, D]), ("Wo", [depth, D, D]),
                    ("W1", [depth, D, FF]), ("W2", [depth, FF, D]),
                    ("ln1g", [depth, D]), ("ln1b", [depth, D]),
                    ("ln2g", [depth, D]), ("ln2b", [depth, D]),
                    ("bo", [depth, D]), ("b1", [depth, FF]),
                    ("b2", [depth, D]), ("maskd", [P, 256]),
                    ("segw", [P, cfg.segs])):
        dt_ = MMDT if nm in wnames else F32
        io[nm] = nc.dram_tensor(nm, shp, dt_, kind="ExternalInput").ap()
    io["yT"] = nc.dram_tensor("yT", [D, R], F32, kind="ExternalOutput").ap()

    rg = [list(range(g * cfg.segs, (g + 1) * cfg.segs)) for g in range(cfg.B)]

    with tile.TileContext(nc) as tc:
        _body(tc, cfg, io, rg)
    nc.compile()
    return nc


def _body(tc, cfg: Cfg, io, rg):
    nc = tc.nc
    D, H, FF, R, P = cfg.D, cfg.H, cfg.FF, cfg.R, cfg.P
    KT, NB, FB, dh = cfg.KT, cfg.NB, cfg.FB, cfg.dh
    HPT = 2                      # heads per 128-partition tile
    HH = H // HPT
    VW = D + H                   # v row-major block width: H slots of (dh+1)
    SW = HH * (dh + 1)           # per-partition-half state width

    MMDT = BF16 if cfg.act_bf16 else (F32R if cfg.use_f32r else F32)
    MVDT = MMDT
    from concourse.tile import add_dep_helper

    import contextlib
    ctx = contextlib.ExitStack()
    cpool = ctx.enter_context(tc.tile_pool(name="cpool", bufs=1))
    xpool = ctx.enter_context(tc.tile_pool(name="xpool", bufs=1))
    apool = ctx.enter_context(tc.tile_pool(name="apool", bufs=1))
    hpool = ctx.enter_context(tc.tile_pool(name="hpool", bufs=2))
    spool = ctx.enter_context(tc.tile_pool(name="spool", bufs=2))
    wpool = ctx.enter_context(tc.tile_pool(name="wpool", bufs=12))
    w1pool = ctx.enter_context(tc.tile_pool(name="w1pool", bufs=16))
    w2pool = ctx.enter_context(tc.tile_pool(name="w2pool", bufs=8))
    ampool = ctx.enter_context(tc.tile_pool(name="ampool", bufs=4))
    dpool = ctx.enter_context(tc.tile_pool(name="dpool", bufs=1, space="DRAM"))
    pmm = ctx.enter_context(tc.tile_pool(name="pmm", bufs=3, space="PSUM"))
    psm = ctx.enter_context(tc.tile_pool(name="psm", bufs=1, space="PSUM"))
    npool = ctx.enter_context(tc.tile_pool(name="npool", bufs=3, space="PSUM"))
    use_warm = cfg.use_f32r and not cfg.act_bf16 and cfg.warm_every
    if use_warm:
        wrmp = ctx.enter_context(tc.tile_pool(name="wrmp", bufs=1,
                                              space="PSUM"))

    identF = cpool.tile([P, P], F32, name="identF")
    make_identity(nc, identF)
    ident = cpool.tile([P, P], MVDT, name="ident")
    nc.vector.tensor_copy(out=ident, in_=identF)
    mask = cpool.tile([P, 256], F32, name="mask")
    nc.sync.dma_start(out=mask, in_=io["maskd"])

    # residual stream x^T: KT tiles of (128, R) packed as (128, KT*R)
    # loaded tile-by-tile so the LN1 partial sums can start on tile 0/1
    # while the rest is still in flight
    x = xpool.tile([P, KT * R], F32, name="x")
    for t in range(KT):
        nc.sync.dma_start(out=x[:, t * R:(t + 1) * R],
                          in_=io["xT"][t * P:(t + 1) * P, :])

    segw = cpool.tile([P, cfg.segs], F32, name="segw")
    nc.sync.dma_start(out=segw, in_=io["segw"])
    epscol = cpool.tile([P, 1], F32, name="epscol")
    nc.vector.memset(epscol, 1e-5)
    onescol = cpool.tile([P, 1], MVDT, name="onescol")
    nc.vector.memset(onescol, 1.0)
    onesrow = cpool.tile([1, P], MVDT, name="onesrow")
    nc.vector.memset(onesrow, 1.0)

    # HAM "warmer": fp32r matmuls don't register as PE activity, so the
    # clock gate throttles to 1.2 GHz mid-kernel.  Interleave a tiny bf16
    # matmul every few real matmuls to keep the activity monitor busy.
    _wst = {"n": 0, "pend": None}
    if use_warm:
        wa = cpool.tile([P, 1], BF16, name="warm_a")
        nc.vector.memset(wa, 1.0)
        wb = cpool.tile([P, cfg.warm_cols], BF16, name="warm_b")
        nc.vector.memset(wb, 0.5)

    def mm(*a, **kw):
        inst = nc.tensor.matmul(*a, **kw)
        if not use_warm:
            return inst
        if _wst["pend"] is not None:
            add_dep_helper(inst.ins, _wst["pend"].ins, False, "warm-order")
            _wst["pend"] = None
        _wst["n"] += 1
        if _wst["n"] % cfg.warm_every == 0:
            wps = wrmp.tile([1, cfg.warm_cols], F32, name="warm_ps",
                            tag="warmps")
            w = nc.tensor.matmul(wps, wa, wb, start=True, stop=True)
            add_dep_helper(w.ins, inst.ins, False, "warm-order")
            _wst["pend"] = w
        return inst

    def pe_keepalive(n, tag):
        """Throwaway f32 matmuls that keep the PE activity monitor busy
        across an otherwise PE-idle stretch (so the clock gate stays at
        8/8).  Nothing ever reads the outputs."""
        for wi in range(n):
            dps = pmm.tile([P, R], F32, name=f"keep_{tag}{wi}", tag="mmps")
            mm(dps[:, 0:256], identF, mask, start=True, stop=True)

    def layer_norm(gcol, bcol, tag):
        """x -> xn (fresh tile). gcol/bcol: (128, KT) column tiles."""
        pe_keepalive(6, tag)
        xn = apool.tile([P, KT * R], MVDT, name=f"xn_{tag}", tag="xn")
        # tree-reduce the feature-tile partial sums (keeps the serial
        # dependency depth at ~3 vector ops instead of KT)
        pa = spool.tile([P, R], F32, name=f"lnpa_{tag}", tag="lnpa", bufs=1)
        pb = spool.tile([P, R], F32, name=f"lnpb_{tag}", tag="lnpb", bufs=1)
        acc = spool.tile([P, R], MVDT, name=f"lnacc_{tag}", tag="lnacc",
                         bufs=1)
        nc.vector.tensor_tensor(out=pa, in0=x[:, 0:R], in1=x[:, R:2 * R],
                                op=OP.add)
        nc.vector.tensor_tensor(out=pb, in0=x[:, 2 * R:3 * R],
                                in1=x[:, 3 * R:4 * R], op=OP.add)
        nc.vector.tensor_tensor(out=acc, in0=pa, in1=pb, op=OP.add)
        nc.vector.tensor_tensor(out=pa, in0=x[:, 4 * R:5 * R],
                                in1=x[:, 5 * R:6 * R], op=OP.add)
        nc.vector.tensor_tensor(out=pb, in0=x[:, 6 * R:7 * R],
                                in1=x[:, 7 * R:8 * R], op=OP.add)
        nc.vector.tensor_tensor(out=pa, in0=pa, in1=pb, op=OP.add)
        nc.vector.tensor_tensor(out=acc, in0=acc, in1=pa, op=OP.add)
        # squared partial sums: squares on ScalarE, pair adds on VectorE
        qa = spool.tile([P, R], F32, name=f"lnqa_{tag}", tag="lnqa", bufs=1)
        qb = spool.tile([P, R], F32, name=f"lnqb_{tag}", tag="lnqb", bufs=1)
        qc = spool.tile([P, R], F32, name=f"lnqc_{tag}", tag="lnqc", bufs=1)
        qd = spool.tile([P, R], F32, name=f"lnqd_{tag}", tag="lnqd", bufs=1)
        sq = spool.tile([P, R], MVDT, name=f"lnsq_{tag}", tag="lnsq", bufs=1)
        sq2 = spool.tile([P, R], F32, name=f"lnsq2_{tag}", tag="lnsq2", bufs=1)
        nc.scalar.activation(qa, x[:, 0:R], AF.Square)
        nc.scalar.activation(qb, x[:, R:2 * R], AF.Square)
        nc.vector.tensor_tensor(out=sq, in0=qa, in1=qb, op=OP.add)
        nc.scalar.activation(qc, x[:, 2 * R:3 * R], AF.Square)
        nc.scalar.activation(qd, x[:, 3 * R:4 * R], AF.Square)
        nc.vector.tensor_tensor(out=sq2, in0=qc, in1=qd, op=OP.add)
        nc.vector.tensor_tensor(out=sq, in0=sq, in1=sq2, op=OP.add)
        nc.scalar.activation(qa, x[:, 4 * R:5 * R], AF.Square)
        nc.scalar.activation(qb, x[:, 5 * R:6 * R], AF.Square)
        nc.vector.tensor_tensor(out=sq2, in0=qa, in1=qb, op=OP.add)
        nc.scalar.activation(qc, x[:, 6 * R:7 * R], AF.Square)
        nc.scalar.activation(qd, x[:, 7 * R:8 * R], AF.Square)
        nc.vector.tensor_tensor(out=qc, in0=qc, in1=qd, op=OP.add)
        nc.vector.tensor_tensor(out=sq2, in0=sq2, in1=qc, op=OP.add)
        nc.vector.tensor_tensor(out=sq, in0=sq, in1=sq2, op=OP.add)
        # partition-direction reduce via PE (ones-vector matmul): keeps the
        # PE active through the LN phase (no HAM cool-down) and is much
        # faster than gpsimd partition_all_reduce
        sum_ps = pmm.tile([P, R], F32, name=f"lnsum_{tag}", tag="mmps")
        sq_ps = pmm.tile([P, R], F32, name=f"lnsq_{tag}", tag="mmps")
        mm(sum_ps[0:1, :], onescol, acc, start=True, stop=True)
        mm(sq_ps[0:1, :], onescol, sq, start=True, stop=True)
        mrow = spool.tile([1, R], F32, name=f"lnmr_{tag}", tag="lnmr", bufs=1)
        irow = spool.tile([1, R], F32, name=f"lnir_{tag}", tag="lnir", bufs=1)
        vrow = spool.tile([1, R], F32, name=f"lnvr_{tag}", tag="lnvr", bufs=1)
        nc.scalar.activation(mrow, sum_ps[0:1, :], AF.Copy, scale=1.0 / D)
        nc.vector.tensor_tensor(out=vrow, in0=mrow, in1=mrow, op=OP.mult)
        nc.vector.scalar_tensor_tensor(out=vrow, in0=sq_ps[0:1, :],
                                       scalar=1.0 / D, in1=vrow,
                                       op0=OP.mult, op1=OP.subtract)
        nc.scalar.activation(vrow, vrow, AF.Sqrt, bias=epscol[0:1, :])
        nc.vector.reciprocal_approx_fast(out=irow, in_=vrow)
        # broadcast mean / istd across partitions via PE (ones-column matmul)
        mrowb = spool.tile([1, R], MVDT, name=f"lnmrb_{tag}", tag="lnmrb",
                           bufs=1)
        irowb = spool.tile([1, R], MVDT, name=f"lnirb_{tag}", tag="lnirb",
                           bufs=1)
        nc.vector.tensor_copy(out=mrowb, in_=mrow)
        nc.vector.tensor_copy(out=irowb, in_=irow)
        m = npool.tile([P, R], F32, name=f"lnm_{tag}", tag="nps")
        istd = npool.tile([P, R], F32, name=f"lnistd_{tag}", tag="nps")
        mm(m, onesrow, mrowb, start=True, stop=True)
        mm(istd, onesrow, irowb, start=True, stop=True)
        for t in range(KT):
            xt = x[:, t * R:(t + 1) * R]
            xnt = xn[:, t * R:(t + 1) * R]
            tmp = spool.tile([P, R], F32, name=f"lntmp_{tag}_{t}", tag="lntmp",
                             bufs=1)
            nc.vector.tensor_tensor(out=tmp, in0=xt, in1=m, op=OP.subtract)
            nc.vector.scalar_tensor_tensor(out=tmp, in0=tmp,
                                           scalar=gcol[:, t:t + 1], in1=istd,
                                           op0=OP.mult, op1=OP.mult)
            nc.scalar.activation(xnt, tmp, AF.Identity, bias=bcol[:, t:t + 1])
        return xn

    def load_strips(w, L, n, tag):
        ss = []
        for t_i in range(n):
            s_ = wpool.tile([P, D], MMDT, name=f"{tag}{t_i}_{L}", tag="wstrip")
            nc.sync.dma_start(out=s_, in_=w[L, t_i * P:(t_i + 1) * P, :])
            ss.append(s_)
        return ss

    # warm the PE clock during the initial x/weight DMA wait
    pe_keepalive(10, "boot")

    for L in range(cfg.depth):
        # per-layer bias/gain columns: (128, KT) / (128, FB)
        cols = {}
        for nm, width in (("ln1g", KT), ("ln1b", KT), ("ln2g", KT),
                          ("ln2b", KT), ("bo", KT), ("b2", KT), ("b1", FB)):
            t_ = spool.tile([P, width], F32, name=f"{nm}c{L}", tag=f"{nm}c")
            nc.sync.dma_start(out=t_, in_=io[nm][L].rearrange("(a p) -> p a",
                                                              p=P))
            cols[nm] = t_

        # ================= attention block =================
        xn = layer_norm(cols["ln1g"], cols["ln1b"], f"l{L}a")

        # --- k^T projection, exp fused on the PSUM->SBUF copyout ---
        ek = apool.tile([P, KT * R], MVDT, name=f"ek{L}", tag="ek")
        kstrips = load_strips(io["Wk"], L, KT, "Wk")
        for p in range(KT):
            ps = pmm.tile([P, R], F32, name=f"kproj_ps{L}", tag="mmps")
            for t_i in range(KT):
                mm(ps, kstrips[t_i][:, p * P:(p + 1) * P],
                   xn[:, t_i * R:(t_i + 1) * R],
                   start=(t_i == 0), stop=(t_i == KT - 1))
            nc.scalar.activation(ek[:, p * R:(p + 1) * R], ps, AF.Exp)

        # --- ek row-major via PE transposes: ekrm (128, NB*D) ---
        ekrm = apool.tile([P, NB * D], MVDT, name=f"ekrm{L}", tag="ekrm")
        for p in range(KT):
            for nb in range(NB):
                tps = psm.tile([P, P], MVDT, name=f"tps{L}", tag="tps")
                nc.tensor.transpose(
                    tps, ek[:, p * R + nb * P:p * R + (nb + 1) * P], ident)
                nc.vector.tensor_copy(
                    out=ekrm[:, nb * D + p * P:nb * D + (p + 1) * P],
                    in_=tps)

        # --- v row-major with interleaved ones columns: (128, NB*VW) ---
        vo = apool.tile([P, NB * VW], MVDT, name=f"vo{L}", tag="vo")
        vo3 = vo.rearrange("p (a c) -> p a c", c=dh + 1)
        ones_src = nc.const_aps.tensor(1.0, (P, NB * H, 1), F32)
        nc.vector.tensor_copy(out=vo3[:, :, dh:dh + 1], in_=ones_src)
        vstrips = load_strips(io["Wv"], L, KT, "Wv")
        nhalf = D // 512 if D >= 512 else 1
        hw = D // nhalf              # dk columns per half (<=512)
        nh = hw // dh                # heads per half
        for nb in range(NB):
            for half in range(nhalf):
                ps = pmm.tile([P, hw], F32, name=f"v_ps{L}", tag="mmps")
                for t_i in range(KT):
                    mm(ps,
                       xn[:, t_i * R + nb * P:t_i * R + (nb + 1) * P],
                       vstrips[t_i][:, half * hw:(half + 1) * hw],
                       start=(t_i == 0), stop=(t_i == KT - 1))
                dst = vo3[:, nb * H + half * nh:nb * H + (half + 1) * nh,
                          0:dh]
                nc.scalar.activation(dst,
                                     ps.rearrange("p (h d) -> p h d", d=dh),
                                     AF.Copy)

        # --- local attention state per head: S_seg = sum_j ek_j^T [v_j|1]
        spack = spool.tile([P, SW], F32, name=f"spack{L}", tag="spack",
                           bufs=1)
        for h in range(H):
            hp, hh = h % HPT, h // HPT
            sps = psm.tile([dh, dh + 1], F32, name=f"s_ps{L}", tag="sps")
            for nb in range(NB):
                ek_s = ekrm[:, nb * D + h * dh:nb * D + (h + 1) * dh]
                vo_s = vo[:, nb * VW + h * (dh + 1):nb * VW + (h + 1) * (dh + 1)]
                if not cfg.act_bf16:
                    ek_s, vo_s = ek_s.bitcast(F32), vo_s.bitcast(F32)
                mm(sps, ek_s, vo_s, start=(nb == 0), stop=(nb == NB - 1))
            nc.vector.tensor_copy(
                out=spack[hp * dh:(hp + 1) * dh,
                          hh * (dh + 1):(hh + 1) * (dh + 1)],
                in_=sps)

        # --- AllGather segment states across this batch's 4 cores ---
        # launched here so it overlaps the q projection + local scores
        ag_in = dpool.tile([H * dh, dh + 1], F32, name=f"agin{L}", tag="agin")
        ag_out = dpool.tile([cfg.segs * H * dh, dh + 1], F32,
                            name=f"agout{L}", tag="agout")
        agv = ag_in.rearrange("(hh hp d) c -> hp d hh c", hp=HPT, d=dh)
        for hp in range(HPT):
            nc.sync.dma_start(
                out=agv[hp],
                in_=spack[hp * dh:(hp + 1) * dh, :]
                .rearrange("p (a c) -> p a c", c=dh + 1))
        nc.gpsimd.collective_compute(
            "AllGather", OP.bypass, replica_groups=rg,
            ins=[ag_in.opt()], outs=[ag_out.opt()])
        sall = spool.tile([P, cfg.segs * SW], F32, name=f"sall{L}", tag="sall",
                          bufs=1)
        agov = ag_out.rearrange("(s hh hp d) c -> s hp d hh c", hp=HPT, d=dh,
                                s=cfg.segs)
        for s in range(cfg.segs):
            for hp in range(HPT):
                nc.sync.dma_start(
                    out=sall[hp * dh:(hp + 1) * dh, s * SW:(s + 1) * SW]
                    .rearrange("p (a c) -> p a c", c=dh + 1),
                    in_=agov[s, hp])
        sinit = spool.tile([P, SW], MVDT, name=f"sinit{L}", tag="sinit",
                           bufs=1)
        nc.vector.tensor_scalar_mul(sinit, sall[:, 0:SW], segw[:, 0:1])
        for s in range(1, cfg.segs):
            nc.vector.scalar_tensor_tensor(
                out=sinit, in0=sall[:, s * SW:(s + 1) * SW],
                scalar=segw[:, s:s + 1], in1=sinit, op0=OP.mult, op1=OP.add)

        # --- q^T projection (overlaps the AllGather) ---
        eq = apool.tile([P, KT * R], MVDT, name=f"eq{L}", tag="eq")
        qstrips = load_strips(io["Wq"], L, KT, "Wq")
        for p in range(KT):
            ps = pmm.tile([P, R], F32, name=f"qproj_ps{L}", tag="mmps")
            for t_i in range(KT):
                mm(ps, qstrips[t_i][:, p * P:(p + 1) * P],
                   xn[:, t_i * R:(t_i + 1) * R],
                   start=(t_i == 0), stop=(t_i == KT - 1))
            nc.scalar.activation(eq[:, p * R:(p + 1) * R], ps, AF.Exp)

        # prefetch Wo strips during the per-head loop
        ostrips = load_strips(io["Wo"], L, KT, "Wo")

        # --- per-head causal attention, processed in hp-pairs so the two
        # heads' 64-row score matmuls occupy distinct PE row-groups and run
        # concurrently.  Phase A computes scores + local numerators for ALL
        # heads (collective-independent, copied straight out of PSUM); phase
        # B adds the prefix-state contribution and divides.  This keeps the
        # PE streaming through the whole AllGather latency. ---
        aT = apool.tile([P, KT * R], MVDT, name=f"aT{L}", tag="aT")
        numers = {}
        for hh in range(HH):
            pair = (2 * hh, 2 * hh + 1)
            amt = {}
            for h in pair:
                amt[h] = ampool.tile([P, NB * 512], MVDT, name=f"am{L}",
                                     tag="am")
            for jb in range(NB):
                last = jb == NB - 1
                istart = jb * P if not last else max(0, R - 256)
                ncols = R - istart
                for h in pair:
                    pb = (h % HPT) * dh
                    eqh = eq[pb:pb + dh, hh * R:(hh + 1) * R]   # (64, R)
                    ekh = ek[pb:pb + dh, hh * R:(hh + 1) * R]
                    am = amt[h]
                    aps = pmm.tile([P, 512], F32, name=f"a_ps{L}", tag="mmps")
                    mm(aps[:, 0:ncols],
                       ekh[:, jb * P:(jb + 1) * P],
                       eqh[:, istart:R], start=True, stop=True)
                    if not last:
                        nc.vector.tensor_tensor(
                            out=am[:, jb * 512 + jb * P:
                                   jb * 512 + (jb + 1) * P],
                            in0=aps[:, 0:P], in1=mask[:, 128:256], op=OP.mult)
                        nc.scalar.activation(
                            am[:, jb * 512 + (jb + 1) * P:jb * 512 + R],
                            aps[:, P:ncols], AF.Copy)
                    else:
                        mw = min(ncols, 256)
                        nc.vector.tensor_tensor(
                            out=am[:, jb * 512 + R - mw:jb * 512 + R],
                            in0=aps[:, ncols - mw:ncols],
                            in1=mask[:, 256 - mw:256], op=OP.mult)
            for h in pair:
                am = amt[h]
                nps = npool.tile([P, R], F32, name=f"n_ps{L}", tag="nps")
                for jb in range(NB):
                    jstart = jb * P if jb < NB - 1 else max(0, R - 256)
                    mm(nps[0:dh + 1, jstart:R],
                       vo[:, jb * VW + h * (dh + 1):
                          jb * VW + (h + 1) * (dh + 1)],
                       am[:, jb * 512 + jstart:jb * 512 + R],
                       start=(jb == 0), stop=(jb == NB - 1))
                numer = spool.tile([dh + 1, R], MVDT, name=f"numer{L}_{h}",
                                   tag="numer", bufs=H)
                numers[h] = numer
                nc.vector.tensor_copy(out=numer, in_=nps[0:dh + 1, :])
        # phase B: prefix contribution + divide (gated on the AllGather)
        for h in range(H):
            hh, pb = h // HPT, (h % HPT) * dh
            eqh = eq[pb:pb + dh, hh * R:(hh + 1) * R]
            pps = npool.tile([P, R], F32, name=f"p_ps{L}", tag="nps")
            mm(pps[0:dh + 1, :],
               sinit[pb:pb + dh, hh * (dh + 1):(hh + 1) * (dh + 1)],
               eqh, start=True, stop=True)
            numer = numers[h]
            nc.vector.tensor_tensor(out=numer, in0=numer,
                                    in1=pps[0:dh + 1, :], op=OP.add)
            dsb = spool.tile([1, R], F32, name=f"dsb{L}", tag="dsb", bufs=1)
            nc.scalar.activation(dsb, numer[dh:dh + 1, :], AF.Copy)
            drow = spool.tile([1, R], F32, name=f"drow{L}", tag="drow",
                              bufs=1)
            nc.vector.reciprocal_approx_fast(out=drow, in_=dsb)
            drowb = spool.tile([1, R], MVDT, name=f"drowb{L}", tag="drowb",
                               bufs=2)
            nc.vector.tensor_copy(out=drowb, in_=drow)
            # broadcast 1/denom across the head's 64 partitions via PE
            dbc = npool.tile([P, R], F32, name=f"dbc_ps{L}", tag="nps")
            mm(dbc[0:dh, :], onesrow[0:1, 0:dh], drowb, start=True, stop=True)
            nc.vector.tensor_tensor(
                out=aT[pb:pb + dh, hh * R:(hh + 1) * R],
                in0=numer[0:dh, :], in1=dbc[0:dh, :], op=OP.mult)

        # --- Wo + residual ---
        for p in range(KT):
            ps = pmm.tile([P, R], F32, name=f"wo_ps{L}", tag="mmps")
            for t_i in range(KT):
                mm(ps, ostrips[t_i][:, p * P:(p + 1) * P],
                   aT[:, t_i * R:(t_i + 1) * R],
                   start=(t_i == 0), stop=(t_i == KT - 1))
            xp = x[:, p * R:(p + 1) * R]
            nc.vector.scalar_tensor_tensor(out=xp, in0=ps,
                                           scalar=cols["bo"][:, p:p + 1],
                                           in1=xp, op0=OP.add, op1=OP.add)

        # ================= FFN block =================
        xn2 = layer_norm(cols["ln2g"], cols["ln2b"], f"l{L}f")
        NE = FF // 512
        for e in range(NE):
            w1s = []
            for t_i in range(KT):
                s_ = w1pool.tile([P, 512], MMDT, name=f"W1s{L}", tag="w1strip")
                nc.sync.dma_start(
                    out=s_, in_=io["W1"][L, t_i * P:(t_i + 1) * P,
                                         e * 512:(e + 1) * 512])
                w1s.append(s_)
            he = hpool.tile([P, 4 * R], MVDT, name=f"he{L}", tag="he")
            for blk in range(4):
                fb = 4 * e + blk
                ps = pmm.tile([P, R], F32, name=f"w1_ps{L}", tag="mmps")
                for t_i in range(KT):
                    mm(ps, w1s[t_i][:, blk * P:(blk + 1) * P],
                       xn2[:, t_i * R:(t_i + 1) * R],
                       start=(t_i == 0), stop=(t_i == KT - 1))
                nc.scalar.activation(he[:, blk * R:(blk + 1) * R], ps, AF.Gelu,
                                     bias=cols["b1"][:, fb:fb + 1])
            w2s = []
            for tt in range(4):
                s_ = w2pool.tile([P, D], MMDT, name=f"W2s{L}", tag="w2strip")
                nc.sync.dma_start(
                    out=s_,
                    in_=io["W2"][L, e * 512 + tt * P:e * 512 + (tt + 1) * P, :])
                w2s.append(s_)
            for p in range(KT):
                ps = pmm.tile([P, R], F32, name=f"w2_ps{L}", tag="mmps")
                for tt in range(4):
                    mm(ps, w2s[tt][:, p * P:(p + 1) * P],
                       he[:, tt * R:(tt + 1) * R],
                       start=(tt == 0), stop=(tt == 3))
                xp = x[:, p * R:(p + 1) * R]
                if e == 0:
                    nc.vector.scalar_tensor_tensor(
                        out=xp, in0=ps, scalar=cols["b2"][:, p:p + 1], in1=xp,
                        op0=OP.add, op1=OP.add)
                else:
                    nc.vector.tensor_tensor(out=xp, in0=xp, in1=ps, op=OP.add)

    for t in range(KT):
        nc.sync.dma_start(out=io["yT"][t * P:(t + 1) * P, :],
                          in_=x[:, t * R:(t + 1) * R])
    ctx.close()


# ----------------------------------------------------------------------------
_BUILT = {}


def _get_program(cfg: Cfg):
    key = (cfg.D, cfg.H, cfg.FF, cfg.R, cfg.depth, cfg.n_cores, cfg.use_f32r,
           cfg.act_bf16, cfg.warm_every, cfg.warm_cols)
    if key not in _BUILT:
        _BUILT[key] = build_program(cfg)
    return _BUILT[key]


def make_in_maps(cfg: Cfg, inputs):
    if cfg.act_bf16:
        import ml_dtypes
        wdt = ml_dtypes.bfloat16
    else:
        wdt = np.float32
    mask = np.zeros((cfg.P, 256), np.float32)
    jj = np.arange(cfg.P)[:, None]
    cc = np.arange(128)[None, :]
    mask[:, 128:256] = (jj <= cc).astype(np.float32)
    shared = dict(
        maskd=mask,
        Wq=np.ascontiguousarray(inputs["Wq"], dtype=wdt),
        Wk=np.ascontiguousarray(inputs["Wk"], dtype=wdt),
        Wv=np.ascontiguousarray(inputs["Wv"], dtype=wdt),
        Wo=np.ascontiguousarray(inputs["Wo"], dtype=wdt),
        W1=np.ascontiguousarray(inputs["W1"], dtype=wdt),
        W2=np.ascontiguousarray(inputs["W2"], dtype=wdt),
        ln1g=np.ascontiguousarray(inputs["ln1_g"], dtype=np.float32),
        ln1b=np.ascontiguousarray(inputs["ln1_b"], dtype=np.float32),
        ln2g=np.ascontiguousarray(inputs["ln2_g"], dtype=np.float32),
        ln2b=np.ascontiguousarray(inputs["ln2_b"], dtype=np.float32),
        bo=np.ascontiguousarray(inputs["bo"], dtype=np.float32),
        b1=np.ascontiguousarray(inputs["b1"], dtype=np.float32),
        b2=np.ascontiguousarray(inputs["b2"], dtype=np.float32),
    )
    x = np.asarray(inputs["x"], dtype=np.float32)
    in_maps = []
    for c in range(cfg.n_cores):
        b, s = c // cfg.segs, c % cfg.segs
        seg_w = np.zeros((cfg.P, cfg.segs), np.float32)
        seg_w[:, :s] = 1.0
        m = dict(shared)
        m["xT"] = np.ascontiguousarray(x[b, s * cfg.R:(s + 1) * cfg.R, :].T)
        m["segw"] = seg_w
        in_maps.append(m)
    return in_maps


def run(cfg: Cfg, inputs, trace=False, **kw):
    nc = _get_program(cfg)
    in_maps = make_in_maps(cfg, inputs)
    res = run_bass_kernel_spmd(nc, in_maps, core_ids=list(range(cfg.n_cores)),
                               trace=trace, **kw)
    B, N = cfg.B, cfg.segs * cfg.R
    out = np.empty((B, N, cfg.D), np.float32)
    for c in range(cfg.n_cores):
        b, s = c // cfg.segs, c % cfg.segs
        out[b, s * cfg.R:(s + 1) * cfg.R, :] = res.results[c]["yT"].T
    return out, res


def kernel(**inputs) -> np.ndarray:
    cfg = Cfg()
    out, _ = run(cfg, inputs)
    return out


# revision 36
# speedup vs baseline: 1.1473x; 1.1473x over previous
"""Trainium2 Bass kernel for a 2-layer linear-attention transformer.

Sharding: 8 cores = 2 batches x 4 sequence segments (512 rows each).
Each core runs the full per-token pipeline on its rows; the only
cross-core dependency is the causal linear-attention prefix state,
exchanged once per layer via a 4-rank AllGather.

On-chip layout: activations are kept feature-major ("transposed",
feature dim on SBUF partitions) so every matmul contracts the partition
dim with no activation transposes.

Schedule (v1): per layer the k/v projections, per-head prefix states and
the AllGather all run BEFORE the q projection, so the collective is
hidden under the q projection + local score matmuls.  The per-head
numerators accumulate in a dedicated PSUM pool so heads pipeline across
the prefix-state wait.  Weight strips double-buffer across phases.
"""

import sys

for _p in ("/opt/trn_rl_repo", "/root/.axon_site/_ro/trn_rl_repo"):
    if _p not in sys.path:
        sys.path.append(_p)

import numpy as np

import concourse.bass as bass
import concourse.mybir as mybir
import concourse.tile as tile
from concourse import bacc, bass_isa
from concourse.bass_utils import run_bass_kernel_spmd
from concourse.masks import make_identity

F32 = mybir.dt.float32
F32R = mybir.dt.float32r
BF16 = mybir.dt.bfloat16
AF = mybir.ActivationFunctionType
OP = mybir.AluOpType


class Cfg:
    def __init__(self, D=1024, H=16, FF=4096, R=512, depth=2, n_cores=8, segs=4,
                 use_f32r=True, act_bf16=True, warm_every=0, warm_cols=512):
        self.D, self.H, self.FF, self.R, self.depth = D, H, FF, R, depth
        self.n_cores, self.segs = n_cores, segs
        self.B = n_cores // segs
        self.dh = D // H
        self.P = 128
        self.KT = D // 128          # k-tiles over D
        self.NB = R // 128          # row blocks per core
        self.FB = FF // 128         # ff blocks
        self.use_f32r = use_f32r
        self.act_bf16 = act_bf16
        self.warm_every = warm_every
        self.warm_cols = warm_cols
        assert self.dh == 64 and self.R % 128 == 0 and self.D % 128 == 0


def build_program(cfg: Cfg):
    """Build the SPMD Bass program."""
    nc = bacc.Bacc("TRN2", target_bir_lowering=False, debug=False,
                   num_devices=cfg.n_cores)
    D, FF, R, P = cfg.D, cfg.FF, cfg.R, cfg.P
    depth = cfg.depth

    MMDT = BF16 if cfg.act_bf16 else (F32R if cfg.use_f32r else F32)
    io = {}
    io["xT"] = nc.dram_tensor("xT", [D, R], F32, kind="ExternalInput").ap()
    wnames = {"Wq", "Wk", "Wv", "Wo", "W1", "W2"}
    for nm, shp in (("Wq", [depth, D, D]), ("Wk", [depth, D, D]),
                    ("Wv", [depth, D, D]), ("Wo", [depth, D, D]),
                    ("W1", [depth, D, FF]), ("W2", [depth, FF, D]),
                    ("ln1g", [depth, D]), ("ln1b", [depth, D]),
                    ("ln2g", [depth, D]), ("ln2b", [depth, D]),
                    ("bo", [depth, D]), ("b1", [depth, FF]),
                    ("b2", [depth, D]), ("maskd", [P, 256]),
                    ("segw", [P, cfg.segs])):
        dt_ = MMDT if nm in wnames else F32
        io[nm] = nc.dram_tensor(nm, shp, dt_, kind="ExternalInput").ap()
    io["yT"] = nc.dram_tensor("yT", [D, R], F32, kind="ExternalOutput").ap()

    rg = [list(range(g * cfg.segs, (g + 1) * cfg.segs)) for g in range(cfg.B)]

    with tile.TileContext(nc) as tc:
        _body(tc, cfg, io, rg)
    nc.compile()
    return nc


def _body(tc, cfg: Cfg, io, rg):
    nc = tc.nc
    D, H, FF, R, P = cfg.D, cfg.H, cfg.FF, cfg.R, cfg.P
    KT, NB, FB, dh = cfg.KT, cfg.NB, cfg.FB, cfg.dh
    HPT = 2                      # heads per 128-partition tile
    HH = H // HPT
    VW = D + H                   # v row-major block width: H slots of (dh+1)
    SW = HH * (dh + 1)           # per-partition-half state width

    MMDT = BF16 if cfg.act_bf16 else (F32R if cfg.use_f32r else F32)
    MVDT = MMDT
    from concourse.tile import add_dep_helper

    import contextlib
    ctx = contextlib.ExitStack()
    cpool = ctx.enter_context(tc.tile_pool(name="cpool", bufs=1))
    xpool = ctx.enter_context(tc.tile_pool(name="xpool", bufs=1))
    apool = ctx.enter_context(tc.tile_pool(name="apool", bufs=1))
    hpool = ctx.enter_context(tc.tile_pool(name="hpool", bufs=2))
    spool = ctx.enter_context(tc.tile_pool(name="spool", bufs=2))
    wpool = ctx.enter_context(tc.tile_pool(name="wpool", bufs=16))
    w1pool = ctx.enter_context(tc.tile_pool(name="w1pool", bufs=16))
    w2pool = ctx.enter_context(tc.tile_pool(name="w2pool", bufs=8))
    ampool = ctx.enter_context(tc.tile_pool(name="ampool", bufs=4))
    dpool = ctx.enter_context(tc.tile_pool(name="dpool", bufs=1, space="DRAM"))
    pmm = ctx.enter_context(tc.tile_pool(name="pmm", bufs=4, space="PSUM"))
    psm = ctx.enter_context(tc.tile_pool(name="psm", bufs=1, space="PSUM"))
    npool = ctx.enter_context(tc.tile_pool(name="npool", bufs=2, space="PSUM"))
    use_warm = cfg.use_f32r and not cfg.act_bf16 and cfg.warm_every
    if use_warm:
        wrmp = ctx.enter_context(tc.tile_pool(name="wrmp", bufs=1,
                                              space="PSUM"))

    identF = cpool.tile([P, P], F32, name="identF")
    make_identity(nc, identF)
    ident = cpool.tile([P, P], MVDT, name="ident")
    nc.vector.tensor_copy(out=ident, in_=identF)
    mask = cpool.tile([P, 256], F32, name="mask")
    nc.sync.dma_start(out=mask, in_=io["maskd"])

    # residual stream x^T: KT tiles of (128, R) packed as (128, KT*R)
    # loaded tile-by-tile so the LN1 partial sums can start on tile 0/1
    # while the rest is still in flight
    x = xpool.tile([P, KT * R], F32, name="x")
    for t in range(KT):
        nc.sync.dma_start(out=x[:, t * R:(t + 1) * R],
                          in_=io["xT"][t * P:(t + 1) * P, :])
    segw = cpool.tile([P, cfg.segs], F32, name="segw")
    nc.sync.dma_start(out=segw, in_=io["segw"])
    epscol = cpool.tile([P, 1], F32, name="epscol")
    nc.vector.memset(epscol, 1e-5)
    onescol = cpool.tile([P, 1], MVDT, name="onescol")
    nc.vector.memset(onescol, 1.0)
    onesrow = cpool.tile([1, P], MVDT, name="onesrow")
    nc.vector.memset(onesrow, 1.0)

    # HAM "warmer": fp32r matmuls don't register as PE activity, so the
    # clock gate throttles to 1.2 GHz mid-kernel.  Interleave a tiny bf16
    # matmul every few real matmuls to keep the activity monitor busy.
    _wst = {"n": 0, "pend": None}
    if use_warm:
        wa = cpool.tile([P, 1], BF16, name="warm_a")
        nc.vector.memset(wa, 1.0)
        wb = cpool.tile([P, cfg.warm_cols], BF16, name="warm_b")
        nc.vector.memset(wb, 0.5)

    def mm(*a, **kw):
        inst = nc.tensor.matmul(*a, **kw)
        if not use_warm:
            return inst
        if _wst["pend"] is not None:
            add_dep_helper(inst.ins, _wst["pend"].ins, False, "warm-order")
            _wst["pend"] = None
        _wst["n"] += 1
        if _wst["n"] % cfg.warm_every == 0:
            wps = wrmp.tile([1, cfg.warm_cols], F32, name="warm_ps",
                            tag="warmps")
            w = nc.tensor.matmul(wps, wa, wb, start=True, stop=True)
            add_dep_helper(w.ins, inst.ins, False, "warm-order")
            _wst["pend"] = w
        return inst

    def pe_keepalive(n, tag):
        """Throwaway f32 matmuls that keep the PE activity monitor busy
        across an otherwise PE-idle stretch (so the clock gate stays at
        8/8).  Nothing ever reads the outputs."""
        for wi in range(n):
            dps = pmm.tile([P, R], F32, name=f"keep_{tag}{wi}", tag="mmps")
            mm(dps[:, 0:256], identF, mask, start=True, stop=True)

    def layer_norm(gcol, bcol, tag):
        """x -> xn (fresh tile). gcol/bcol: (128, KT) column tiles."""
        pe_keepalive(5, tag)
        xn = apool.tile([P, KT * R], MVDT, name=f"xn_{tag}", tag="xn")
        # tree-reduce the feature-tile partial sums (keeps the serial
        # dependency depth at ~3 vector ops instead of KT)
        pa = spool.tile([P, R], F32, name=f"lnpa_{tag}", tag="lnpa", bufs=1)
        pb = spool.tile([P, R], F32, name=f"lnpb_{tag}", tag="lnpb", bufs=1)
        acc = spool.tile([P, R], MVDT, name=f"lnacc_{tag}", tag="lnacc",
                         bufs=1)
        nc.vector.tensor_tensor(out=pa, in0=x[:, 0:R], in1=x[:, R:2 * R],
                                op=OP.add)
        nc.vector.tensor_tensor(out=pb, in0=x[:, 2 * R:3 * R],
                                in1=x[:, 3 * R:4 * R], op=OP.add)
        nc.vector.tensor_tensor(out=acc, in0=pa, in1=pb, op=OP.add)
        nc.vector.tensor_tensor(out=pa, in0=x[:, 4 * R:5 * R],
                                in1=x[:, 5 * R:6 * R], op=OP.add)
        nc.vector.tensor_tensor(out=pb, in0=x[:, 6 * R:7 * R],
                                in1=x[:, 7 * R:8 * R], op=OP.add)
        nc.vector.tensor_tensor(out=pa, in0=pa, in1=pb, op=OP.add)
        nc.vector.tensor_tensor(out=acc, in0=acc, in1=pa, op=OP.add)
        # squared partial sums: squares on ScalarE, pair adds on VectorE
        qa = spool.tile([P, R], F32, name=f"lnqa_{tag}", tag="lnqa", bufs=1)
        qb = spool.tile([P, R], F32, name=f"lnqb_{tag}", tag="lnqb", bufs=1)
        qc = spool.tile([P, R], F32, name=f"lnqc_{tag}", tag="lnqc", bufs=1)
        qd = spool.tile([P, R], F32, name=f"lnqd_{tag}", tag="lnqd", bufs=1)
        sq = spool.tile([P, R], MVDT, name=f"lnsq_{tag}", tag="lnsq", bufs=1)
        sq2 = spool.tile([P, R], F32, name=f"lnsq2_{tag}", tag="lnsq2", bufs=1)
        nc.scalar.activation(qa, x[:, 0:R], AF.Square)
        nc.scalar.activation(qb, x[:, R:2 * R], AF.Square)
        nc.vector.tensor_tensor(out=sq, in0=qa, in1=qb, op=OP.add)
        nc.scalar.activation(qc, x[:, 2 * R:3 * R], AF.Square)
        nc.scalar.activation(qd, x[:, 3 * R:4 * R], AF.Square)
        nc.vector.tensor_tensor(out=sq2, in0=qc, in1=qd, op=OP.add)
        nc.vector.tensor_tensor(out=sq, in0=sq, in1=sq2, op=OP.add)
        nc.scalar.activation(qa, x[:, 4 * R:5 * R], AF.Square)
        nc.scalar.activation(qb, x[:, 5 * R:6 * R], AF.Square)
        nc.vector.tensor_tensor(out=sq2, in0=qa, in1=qb, op=OP.add)
        nc.scalar.activation(qc, x[:, 6 * R:7 * R], AF.Square)
        nc.scalar.activation(qd, x[:, 7 * R:8 * R], AF.Square)
        nc.vector.tensor_tensor(out=qc, in0=qc, in1=qd, op=OP.add)
        nc.vector.tensor_tensor(out=sq2, in0=sq2, in1=qc, op=OP.add)
        nc.vector.tensor_tensor(out=sq, in0=sq, in1=sq2, op=OP.add)
        # partition-direction reduce via PE (ones-vector matmul): keeps the
        # PE active through the LN phase (no HAM cool-down) and is much
        # faster than gpsimd partition_all_reduce
        sum_ps = pmm.tile([P, R], F32, name=f"lnsum_{tag}", tag="mmps")
        sq_ps = pmm.tile([P, R], F32, name=f"lnsq_{tag}", tag="mmps")
        mm(sum_ps[0:1, :], onescol, acc, start=True, stop=True)
        mm(sq_ps[0:1, :], onescol, sq, start=True, stop=True)
        mrow = spool.tile([1, R], F32, name=f"lnmr_{tag}", tag="lnmr", bufs=1)
        irow = spool.tile([1, R], F32, name=f"lnir_{tag}", tag="lnir", bufs=1)
        vrow = spool.tile([1, R], F32, name=f"lnvr_{tag}", tag="lnvr", bufs=1)
        nc.scalar.activation(mrow, sum_ps[0:1, :], AF.Copy, scale=1.0 / D)
        nc.vector.tensor_tensor(out=vrow, in0=mrow, in1=mrow, op=OP.mult)
        nc.vector.scalar_tensor_tensor(out=vrow, in0=sq_ps[0:1, :],
                                       scalar=1.0 / D, in1=vrow,
                                       op0=OP.mult, op1=OP.subtract)
        nc.scalar.activation(vrow, vrow, AF.Sqrt, bias=epscol[0:1, :])
        nc.vector.reciprocal_approx_fast(out=irow, in_=vrow)
        # broadcast mean / istd across partitions via PE (ones-column matmul)
        mrowb = spool.tile([1, R], MVDT, name=f"lnmrb_{tag}", tag="lnmrb",
                           bufs=1)
        irowb = spool.tile([1, R], MVDT, name=f"lnirb_{tag}", tag="lnirb",
                           bufs=1)
        nc.vector.tensor_copy(out=mrowb, in_=mrow)
        nc.vector.tensor_copy(out=irowb, in_=irow)
        m = npool.tile([P, R], F32, name=f"lnm_{tag}", tag="nps")
        istd = npool.tile([P, R], F32, name=f"lnistd_{tag}", tag="nps")
        mm(m, onesrow, mrowb, start=True, stop=True)
        mm(istd, onesrow, irowb, start=True, stop=True)
        for t in range(KT):
            xt = x[:, t * R:(t + 1) * R]
            xnt = xn[:, t * R:(t + 1) * R]
            tmp = spool.tile([P, R], F32, name=f"lntmp_{tag}_{t}", tag="lntmp",
                             bufs=1)
            nc.vector.tensor_tensor(out=tmp, in0=xt, in1=m, op=OP.subtract)
            nc.vector.scalar_tensor_tensor(out=tmp, in0=tmp,
                                           scalar=gcol[:, t:t + 1], in1=istd,
                                           op0=OP.mult, op1=OP.mult)
            nc.scalar.activation(xnt, tmp, AF.Identity, bias=bcol[:, t:t + 1])
        return xn

    def load_strips(w, L, n, tag):
        ss = []
        for t_i in range(n):
            s_ = wpool.tile([P, D], MMDT, name=f"{tag}{t_i}_{L}", tag="wstrip")
            nc.sync.dma_start(out=s_, in_=w[L, t_i * P:(t_i + 1) * P, :])
            ss.append(s_)
        return ss

    # warm the PE clock during the initial x/weight DMA wait
    pe_keepalive(8, "boot")

    for L in range(cfg.depth):
        # per-layer bias/gain columns: (128, KT) / (128, FB)
        cols = {}
        for nm, width in (("ln1g", KT), ("ln1b", KT), ("ln2g", KT),
                          ("ln2b", KT), ("bo", KT), ("b2", KT), ("b1", FB)):
            t_ = spool.tile([P, width], F32, name=f"{nm}c{L}", tag=f"{nm}c")
            nc.sync.dma_start(out=t_, in_=io[nm][L].rearrange("(a p) -> p a",
                                                              p=P))
            cols[nm] = t_

        # ================= attention block =================
        xn = layer_norm(cols["ln1g"], cols["ln1b"], f"l{L}a")

        # --- k^T projection, exp fused on the PSUM->SBUF copyout ---
        ek = apool.tile([P, KT * R], MVDT, name=f"ek{L}", tag="ek")
        kstrips = load_strips(io["Wk"], L, KT, "Wk")
        for p in range(KT):
            ps = pmm.tile([P, R], F32, name=f"kproj_ps{L}", tag="mmps")
            for t_i in range(KT):
                mm(ps, kstrips[t_i][:, p * P:(p + 1) * P],
                   xn[:, t_i * R:(t_i + 1) * R],
                   start=(t_i == 0), stop=(t_i == KT - 1))
            nc.scalar.activation(ek[:, p * R:(p + 1) * R], ps, AF.Exp)

        # --- ek row-major via PE transposes: ekrm (128, NB*D) ---
        ekrm = apool.tile([P, NB * D], MVDT, name=f"ekrm{L}", tag="ekrm")
        for p in range(KT):
            for nb in range(NB):
                tps = psm.tile([P, P], MVDT, name=f"tps{L}", tag="tps")
                nc.tensor.transpose(
                    tps, ek[:, p * R + nb * P:p * R + (nb + 1) * P], ident)
                nc.vector.tensor_copy(
                    out=ekrm[:, nb * D + p * P:nb * D + (p + 1) * P],
                    in_=tps)

        # --- v row-major with interleaved ones columns: (128, NB*VW) ---
        vo = apool.tile([P, NB * VW], MVDT, name=f"vo{L}", tag="vo")
        vo3 = vo.rearrange("p (a c) -> p a c", c=dh + 1)
        ones_src = nc.const_aps.tensor(1.0, (P, NB * H, 1), F32)
        nc.vector.tensor_copy(out=vo3[:, :, dh:dh + 1], in_=ones_src)
        vstrips = load_strips(io["Wv"], L, KT, "Wv")
        nhalf = D // 512 if D >= 512 else 1
        hw = D // nhalf              # dk columns per half (<=512)
        nh = hw // dh                # heads per half
        for nb in range(NB):
            for half in range(nhalf):
                ps = pmm.tile([P, hw], F32, name=f"v_ps{L}", tag="mmps")
                for t_i in range(KT):
                    mm(ps,
                       xn[:, t_i * R + nb * P:t_i * R + (nb + 1) * P],
                       vstrips[t_i][:, half * hw:(half + 1) * hw],
                       start=(t_i == 0), stop=(t_i == KT - 1))
                dst = vo3[:, nb * H + half * nh:nb * H + (half + 1) * nh,
                          0:dh]
                nc.scalar.activation(dst,
                                     ps.rearrange("p (h d) -> p h d", d=dh),
                                     AF.Copy)

        # --- local attention state per head: S_seg = sum_j ek_j^T [v_j|1]
        spack = spool.tile([P, SW], F32, name=f"spack{L}", tag="spack",
                           bufs=1)
        for h in range(H):
            hp, hh = h % HPT, h // HPT
            sps = psm.tile([dh, dh + 1], F32, name=f"s_ps{L}", tag="sps")
            for nb in range(NB):
                ek_s = ekrm[:, nb * D + h * dh:nb * D + (h + 1) * dh]
                vo_s = vo[:, nb * VW + h * (dh + 1):nb * VW + (h + 1) * (dh + 1)]
                if not cfg.act_bf16:
                    ek_s, vo_s = ek_s.bitcast(F32), vo_s.bitcast(F32)
                mm(sps, ek_s, vo_s, start=(nb == 0), stop=(nb == NB - 1))
            nc.vector.tensor_copy(
                out=spack[hp * dh:(hp + 1) * dh,
                          hh * (dh + 1):(hh + 1) * (dh + 1)],
                in_=sps)

        # --- AllGather segment states across this batch's 4 cores ---
        # launched here so it overlaps the q projection + local scores
        ag_in = dpool.tile([H * dh, dh + 1], F32, name=f"agin{L}", tag="agin")
        ag_out = dpool.tile([cfg.segs * H * dh, dh + 1], F32,
                            name=f"agout{L}", tag="agout")
        agv = ag_in.rearrange("(hh hp d) c -> hp d hh c", hp=HPT, d=dh)
        for hp in range(HPT):
            nc.sync.dma_start(
                out=agv[hp],
                in_=spack[hp * dh:(hp + 1) * dh, :]
                .rearrange("p (a c) -> p a c", c=dh + 1))
        nc.gpsimd.collective_compute(
            "AllGather", OP.bypass, replica_groups=rg,
            ins=[ag_in.opt()], outs=[ag_out.opt()])
        sall = spool.tile([P, cfg.segs * SW], F32, name=f"sall{L}", tag="sall",
                          bufs=1)
        agov = ag_out.rearrange("(s hh hp d) c -> s hp d hh c", hp=HPT, d=dh,
                                s=cfg.segs)
        for s in range(cfg.segs):
            for hp in range(HPT):
                nc.sync.dma_start(
                    out=sall[hp * dh:(hp + 1) * dh, s * SW:(s + 1) * SW]
                    .rearrange("p (a c) -> p a c", c=dh + 1),
                    in_=agov[s, hp])
        sinit = spool.tile([P, SW], MVDT, name=f"sinit{L}", tag="sinit",
                           bufs=1)
        nc.vector.tensor_scalar_mul(sinit, sall[:, 0:SW], segw[:, 0:1])
        for s in range(1, cfg.segs):
            nc.vector.scalar_tensor_tensor(
                out=sinit, in0=sall[:, s * SW:(s + 1) * SW],
                scalar=segw[:, s:s + 1], in1=sinit, op0=OP.mult, op1=OP.add)

        # --- q^T projection (overlaps the AllGather) ---
        eq = apool.tile([P, KT * R], MVDT, name=f"eq{L}", tag="eq")
        qstrips = load_strips(io["Wq"], L, KT, "Wq")
        for p in range(KT):
            ps = pmm.tile([P, R], F32, name=f"qproj_ps{L}", tag="mmps")
            for t_i in range(KT):
                mm(ps, qstrips[t_i][:, p * P:(p + 1) * P],
                   xn[:, t_i * R:(t_i + 1) * R],
                   start=(t_i == 0), stop=(t_i == KT - 1))
            nc.scalar.activation(eq[:, p * R:(p + 1) * R], ps, AF.Exp)

        # prefetch Wo strips during the per-head loop
        ostrips = load_strips(io["Wo"], L, KT, "Wo")

        # --- per-head causal attention, processed in hp-pairs so the two
        # heads' 64-row score matmuls occupy distinct PE row-groups and run
        # concurrently; numerators are copied out of PSUM immediately so the
        # PSUM pool recycles without waiting on the divide epilogue ---
        aT = apool.tile([P, KT * R], MVDT, name=f"aT{L}", tag="aT")
        for hh in range(HH):
            pair = (2 * hh, 2 * hh + 1)
            amt = {}
            for h in pair:
                amt[h] = ampool.tile([P, NB * 512], MVDT, name=f"am{L}",
                                     tag="am")
            for jb in range(NB):
                last = jb == NB - 1
                istart = jb * P if not last else max(0, R - 256)
                ncols = R - istart
                for h in pair:
                    pb = (h % HPT) * dh
                    eqh = eq[pb:pb + dh, hh * R:(hh + 1) * R]   # (64, R)
                    ekh = ek[pb:pb + dh, hh * R:(hh + 1) * R]
                    am = amt[h]
                    aps = pmm.tile([P, 512], F32, name=f"a_ps{L}", tag="mmps")
                    mm(aps[:, 0:ncols],
                       ekh[:, jb * P:(jb + 1) * P],
                       eqh[:, istart:R], start=True, stop=True)
                    if not last:
                        nc.vector.tensor_tensor(
                            out=am[:, jb * 512 + jb * P:
                                   jb * 512 + (jb + 1) * P],
                            in0=aps[:, 0:P], in1=mask[:, 128:256], op=OP.mult)
                        nc.scalar.activation(
                            am[:, jb * 512 + (jb + 1) * P:jb * 512 + R],
                            aps[:, P:ncols], AF.Copy)
                    else:
                        mw = min(ncols, 256)
                        nc.vector.tensor_tensor(
                            out=am[:, jb * 512 + R - mw:jb * 512 + R],
                            in0=aps[:, ncols - mw:ncols],
                            in1=mask[:, 256 - mw:256], op=OP.mult)
            for h in pair:
                pb = (h % HPT) * dh
                eqh = eq[pb:pb + dh, hh * R:(hh + 1) * R]
                am = amt[h]
                nps = npool.tile([P, R], F32, name=f"n_ps{L}", tag="nps")
                for jb in range(NB):
                    jstart = jb * P if jb < NB - 1 else max(0, R - 256)
                    mm(nps[0:dh + 1, jstart:R],
                       vo[:, jb * VW + h * (dh + 1):
                          jb * VW + (h + 1) * (dh + 1)],
                       am[:, jb * 512 + jstart:jb * 512 + R],
                       start=(jb == 0), stop=False)
                mm(nps[0:dh + 1, :],
                   sinit[pb:pb + dh, hh * (dh + 1):(hh + 1) * (dh + 1)],
                   eqh, start=False, stop=True)
                numer = spool.tile([dh, R], F32, name=f"numer{L}",
                                   tag="numer", bufs=2)
                nc.vector.tensor_copy(out=numer, in_=nps[0:dh, :])
                dsb = spool.tile([1, R], F32, name=f"dsb{L}", tag="dsb",
                                 bufs=1)
                nc.scalar.activation(dsb, nps[dh:dh + 1, :], AF.Copy)
                drow = spool.tile([1, R], F32, name=f"drow{L}", tag="drow",
                                  bufs=1)
                nc.vector.reciprocal_approx_fast(out=drow, in_=dsb)
                dbc = spool.tile([P, R], F32, name=f"dbc{L}", tag="dbc",
                                 bufs=1)
                nc.gpsimd.partition_broadcast(dbc[0:dh, :], drow, channels=dh)
                nc.vector.tensor_tensor(
                    out=aT[pb:pb + dh, hh * R:(hh + 1) * R],
                    in0=numer, in1=dbc[0:dh, :], op=OP.mult)

        # --- Wo + residual ---
        for p in range(KT):
            ps = pmm.tile([P, R], F32, name=f"wo_ps{L}", tag="mmps")
            for t_i in range(KT):
                mm(ps, ostrips[t_i][:, p * P:(p + 1) * P],
                   aT[:, t_i * R:(t_i + 1) * R],
                   start=(t_i == 0), stop=(t_i == KT - 1))
            xp = x[:, p * R:(p + 1) * R]
            nc.vector.scalar_tensor_tensor(out=xp, in0=ps,
                                           scalar=cols["bo"][:, p:p + 1],
                                           in1=xp, op0=OP.add, op1=OP.add)

        # ================= FFN block =================
        xn2 = layer_norm(cols["ln2g"], cols["ln2b"], f"l{L}f")
        NE = FF // 512
        for e in range(NE):
            w1s = []
            for t_i in range(KT):
                s_ = w1pool.tile([P, 512], MMDT, name=f"W1s{L}", tag="w1strip")
                nc.sync.dma_start(
                    out=s_, in_=io["W1"][L, t_i * P:(t_i + 1) * P,
                                         e * 512:(e + 1) * 512])
                w1s.append(s_)
            he = hpool.tile([P, 4 * R], MVDT, name=f"he{L}", tag="he")
            for blk in range(4):
                fb = 4 * e + blk
                ps = pmm.tile([P, R], F32, name=f"w1_ps{L}", tag="mmps")
                for t_i in range(KT):
                    mm(ps, w1s[t_i][:, blk * P:(blk + 1) * P],
                       xn2[:, t_i * R:(t_i + 1) * R],
                       start=(t_i == 0), stop=(t_i == KT - 1))
                nc.scalar.activation(he[:, blk * R:(blk + 1) * R], ps, AF.Gelu,
                                     bias=cols["b1"][:, fb:fb + 1])
            w2s = []
            for tt in range(4):
                s_ = w2pool.tile([P, D], MMDT, name=f"W2s{L}", tag="w2strip")
                nc.sync.dma_start(
                    out=s_,
                    in_=io["W2"][L, e * 512 + tt * P:e * 512 + (tt + 1) * P, :])
                w2s.append(s_)
            for p in range(KT):
                ps = pmm.tile([P, R], F32, name=f"w2_ps{L}", tag="mmps")
                for tt in range(4):
                    mm(ps, w2s[tt][:, p * P:(p + 1) * P],
                       he[:, tt * R:(tt + 1) * R],
                       start=(tt == 0), stop=(tt == 3))
                xp = x[:, p * R:(p + 1) * R]
                if e == 0:
                    nc.vector.scalar_tensor_tensor(
                        out=xp, in0=ps, scalar=cols["b2"][:, p:p + 1], in1=xp,
                        op0=OP.add, op1=OP.add)
                else:
                    nc.vector.tensor_tensor(out=xp, in0=xp, in1=ps, op=OP.add)

    for t in range(KT):
        nc.sync.dma_start(out=io["yT"][t * P:(t + 1) * P, :],
                          in_=x[:, t * R:(t + 1) * R])
    ctx.close()


# ----------------------------------------------------------------------------
_BUILT = {}


def _get_program(cfg: Cfg):
    key = (cfg.D, cfg.H, cfg.FF, cfg.R, cfg.depth, cfg.n_cores, cfg.use_f32r,
           cfg.act_bf16, cfg.warm_every, cfg.warm_cols)
    if key not in _BUILT:
        _BUILT[key] = build_program(cfg)
    return _BUILT[key]


def make_in_maps(cfg: Cfg, inputs):
    if cfg.act_bf16:
        import ml_dtypes
        wdt = ml_dtypes.bfloat16
    else:
        wdt = np.float32
    mask = np.zeros((cfg.P, 256), np.float32)
    jj = np.arange(cfg.P)[:, None]
    cc = np.arange(128)[None, :]
    mask[:, 128:256] = (jj <= cc).astype(np.float32)
    shared = dict(
        maskd=mask,
        Wq=np.ascontiguousarray(inputs["Wq"], dtype=wdt),
        Wk=np.ascontiguousarray(inputs["Wk"], dtype=wdt),
        Wv=np.ascontiguousarray(inputs["Wv"], dtype=wdt),
        Wo=np.ascontiguousarray(inputs["Wo"], dtype=wdt),
        W1=np.ascontiguousarray(inputs["W1"], dtype=wdt),
        W2=np.ascontiguousarray(inputs["W2"], dtype=wdt),
        ln1g=np.ascontiguousarray(inputs["ln1_g"], dtype=np.float32),
        ln1b=np.ascontiguousarray(inputs["ln1_b"], dtype=np.float32),
        ln2g=np.ascontiguousarray(inputs["ln2_g"], dtype=np.float32),
        ln2b=np.ascontiguousarray(inputs["ln2_b"], dtype=np.float32),
        bo=np.ascontiguousarray(inputs["bo"], dtype=np.float32),
        b1=np.ascontiguousarray(inputs["b1"], dtype=np.float32),
        b2=np.ascontiguousarray(inputs["b2"], dtype=np.float32),
    )
    x = np.asarray(inputs["x"], dtype=np.float32)
    in_maps = []
    for c in range(cfg.n_cores):
        b, s = c // cfg.segs, c % cfg.segs
        seg_w = np.zeros((cfg.P, cfg.segs), np.float32)
        seg_w[:, :s] = 1.0
        m = dict(shared)
        m["xT"] = np.ascontiguousarray(x[b, s * cfg.R:(s + 1) * cfg.R, :].T)
        m["segw"] = seg_w
        in_maps.append(m)
    return in_maps


def run(cfg: Cfg, inputs, trace=False, **kw):
    nc = _get_program(cfg)
    in_maps = make_in_maps(cfg, inputs)
    res = run_bass_kernel_spmd(nc, in_maps, core_ids=list(range(cfg.n_cores)),
                               trace=trace, **kw)
    B, N = cfg.B, cfg.segs * cfg.R
    out = np.empty((B, N, cfg.D), np.float32)
    for c in range(cfg.n_cores):
        b, s = c // cfg.segs, c % cfg.segs
        out[b, s * cfg.R:(s + 1) * cfg.R, :] = res.results[c]["yT"].T
    return out, res


def kernel(**inputs) -> np.ndarray:
    cfg = Cfg()
    out, _ = run(cfg, inputs)
    return out
